# revision 6
# baseline (speedup 1.0000x reference)
"""Causal self-attention (B=2, T=2048, d_model=1024, H=16) on 8 TRN2 NeuronCores.

Sharding: core c handles batch b = c//4 and head group g = c%4 (heads 4g..4g+3).
Each core computes QKV projection for its heads, causal attention, and a partial
output projection y_partial = attn_out @ Wo[g*256:(g+1)*256, :]. The host sums
the 4 partials per batch (the tensor-parallel all-reduce, done on host).

Layouts on device (per core):
  xT  [1024, 2048]  = x[b].T             (contraction dim on partitions)
  qT/kT [128, 2, 2048]                   (two heads packed per 128 partitions,
                                          head dim 64 on partitions)
  S^T tiles [128 keys, <=512 queries]    softmax over keys happens via PE:
                                         V' = [V | 1] so the PV matmul also
                                         produces per-query denominators.
"""
import sys

sys.path.insert(0, "/opt/trn_rl_repo")

import numpy as np

B, T, C = 2, 2048, 1024
NH_TOT = 16
HD = 64
NH = 4          # heads per core
CO = NH * HD    # 256 channels per core
NCORES = 8
SCALE = 1.0 / 32.0  # d_model ** -0.5

_compiled = None


def _build():
    import concourse.bass as bass  # noqa: F401
    import concourse.mybir as mybir
    import concourse.tile as tile
    from concourse import bacc

    F32 = mybir.dt.float32
    MULT = mybir.AluOpType.mult
    EXP = mybir.ActivationFunctionType.Exp

    nc = bacc.Bacc("TRN2", target_bir_lowering=False)

    xT = nc.declare_dram_parameter("xT", [C, T], F32, isOutput=False)
    wq = nc.declare_dram_parameter("wq", [C, CO], F32, isOutput=False)
    wk = nc.declare_dram_parameter("wk", [C, CO], F32, isOutput=False)
    wv = nc.declare_dram_parameter("wv", [C, CO], F32, isOutput=False)
    wo = nc.declare_dram_parameter("wo", [CO, C], F32, isOutput=False)
    mask = nc.declare_dram_parameter("mask", [128, 128], F32, isOutput=False)
    y = nc.declare_dram_parameter("y", [T, C], F32, isOutput=True)

    xT_t = xT.rearrange("(o p) t -> p o t", p=128)   # [128, 8, 2048]
    wq_t = wq.rearrange("(o p) m -> p o m", p=128)   # [128, 8, 256]
    wk_t = wk.rearrange("(o p) m -> p o m", p=128)
    wv_t = wv.rearrange("(o p) m -> p o m", p=128)
    wo_t = wo.rearrange("(o p) m -> p o m", p=128)   # [128, 2, 1024]

    with tile.TileContext(nc) as tc:
        with (
            tc.tile_pool(name="wpool", bufs=1) as wpool,
            tc.tile_pool(name="qkvpool", bufs=1) as qkvpool,
            tc.tile_pool(name="psa", bufs=2, space="PSUM") as psa,
            tc.tile_pool(name="psb", bufs=2, space="PSUM") as psb,
        ):
            wq_sb = wpool.tile([128, 8, CO], F32, tag="wq")
            wk_sb = wpool.tile([128, 8, CO], F32, tag="wk")
            wv_sb = wpool.tile([128, 8, CO], F32, tag="wv")
            wo_sb = wpool.tile([128, 2, C], F32, tag="wo")
            mask_sb = wpool.tile([128, 128], F32, tag="mask")
            nc.sync.dma_start(wq_sb[:], wq_t[:])
            nc.sync.dma_start(wk_sb[:], wk_t[:])
            nc.sync.dma_start(wv_sb[:], wv_t[:])
            nc.sync.dma_start(wo_sb[:], wo_t[:])
            nc.sync.dma_start(mask_sb[:], mask[:])

            qT_sb = qkvpool.tile([128, 2, T], F32, tag="qT")
            kT_sb = qkvpool.tile([128, 2, T], F32, tag="kT")
            # V' per (t-block, head): 64 cols of V then a ones column
            vp_sb = qkvpool.tile([128, 16, NH, HD + 1], F32, tag="vp")
            nc.vector.memset(vp_sb[:, :, :, HD], 1.0)

            # ---------------- Phase 1: QKV projection ----------------
            with tc.tile_pool(name="xpool", bufs=1) as xpool:
                xT_sb = xpool.tile([128, 8, T], F32, tag="xT")
                for kc in range(8):
                    nc.sync.dma_start(xT_sb[:, kc, :], xT_t[:, kc, :])

                # qT/kT: [c_out pair on partitions, t free]
                for w_sb, dst in ((wq_sb, qT_sb), (wk_sb, kT_sb)):
                    for m in range(2):
                        for t4 in range(4):
                            pq = psa.tile([128, 512], F32, tag="mm")
                            for kc in range(8):
                                nc.tensor.matmul(
                                    pq[:],
                                    w_sb[:, kc, m * 128:(m + 1) * 128],
                                    xT_sb[:, kc, t4 * 512:(t4 + 1) * 512],
                                    start=(kc == 0),
                                    stop=(kc == 7),
                                )
                            nc.vector.tensor_copy(
                                dst[:, m, t4 * 512:(t4 + 1) * 512], pq[:]
                            )

                # V in [t on partitions, head channels] layout
                for tb in range(16):
                    pv = psb.tile([128, CO], F32, tag="v")
                    for kc in range(8):
                        nc.tensor.matmul(
                            pv[:],
                            xT_sb[:, kc, tb * 128:(tb + 1) * 128],
                            wv_sb[:, kc, :],
                            start=(kc == 0),
                            stop=(kc == 7),
                        )
                    nc.vector.tensor_copy(
                        vp_sb[:, tb, :, 0:HD],
                        pv[:].rearrange("p (h d) -> p h d", h=NH),
                    )

            # ---------------- Phase 2: causal attention ----------------
            with (
                tc.tile_pool(name="attnpool", bufs=1) as attnpool,
                tc.tile_pool(name="etpool", bufs=3) as etpool,
                tc.tile_pool(name="stagepool", bufs=2) as stagepool,
                tc.tile_pool(name="bcastpool", bufs=2) as bcastpool,
                tc.tile_pool(name="ypool", bufs=3) as ypool,
            ):
                oT_sb = attnpool.tile([128, 2, T], F32, tag="oT")
                sums_sb = attnpool.tile([NH, T], F32, tag="sums")
                recip_sb = attnpool.tile([NH, T], F32, tag="recip")

                for h in range(NH):
                    po2 = h % 2          # partition offset selector
                    mo2 = h // 2         # m-block
                    q_h = qT_sb[64 * po2:64 * po2 + 64, mo2, :]
                    k_h = kT_sb[64 * po2:64 * po2 + 64, mo2, :]
                    sumstage = stagepool.tile([65, T], F32, tag="sumstage")

                    for ic in range(4):
                        i_base = 512 * ic
                        po = psa.tile([65, 512], F32, tag="o")
                        jb_last = 4 * ic + 3
                        for jb in range(jb_last + 1):
                            i0 = max(i_base, 128 * jb)
                            n = i_base + 512 - i0
                            ps_s = psa.tile([128, 512], F32, tag="s")
                            nc.tensor.matmul(
                                ps_s[:, :n],
                                k_h[:, jb * 128:(jb + 1) * 128],
                                q_h[:, i0:i0 + n],
                                start=True,
                                stop=True,
                            )
                            et = etpool.tile([128, 512], F32, tag="et")
                            nc.scalar.activation(
                                et[:, :n], ps_s[:, :n], EXP, scale=SCALE
                            )
                            if 128 * jb >= i_base:
                                # diagonal block: zero keys below the diagonal
                                nc.vector.tensor_tensor(
                                    et[:, 0:128], et[:, 0:128], mask_sb[:], MULT
                                )
                            nc.tensor.matmul(
                                po[:, i0 - i_base:i0 - i_base + n],
                                vp_sb[:, jb, h, :],
                                et[:, :n],
                                start=(jb == 0),
                                stop=(jb == jb_last),
                            )
                        # unnormalized O^T chunk + denominators
                        nc.vector.tensor_copy(
                            oT_sb[64 * po2:64 * po2 + 64, mo2,
                                  i_base:i_base + 512],
                            po[0:64, :],
                        )
                        nc.vector.tensor_copy(
                            sumstage[64:65, i_base:i_base + 512], po[64:65, :]
                        )
                    # denominators to partition h (cross-partition move)
                    nc.sync.dma_start(sums_sb[h:h + 1, :], sumstage[64:65, :])

                nc.vector.reciprocal(recip_sb[:], sums_sb[:])

                # normalize: oT[head] *= recip[head] broadcast across partitions.
                # Broadcast lands on the same partition range as the head so the
                # DVE multiply has a single start partition.
                for h in range(NH):
                    po2, mo2 = h % 2, h // 2
                    off = 64 * po2
                    bc = bcastpool.tile([128, T], F32, tag="bc")
                    nc.sync.dma_start(
                        bc[off:off + 64, :],
                        recip_sb[h:h + 1, None, :].to_broadcast([1, 64, T]),
                    )
                    o_h = oT_sb[off:off + 64, mo2, :]
                    nc.vector.tensor_tensor(o_h, o_h, bc[off:off + 64, :], MULT)

                # ---------------- Phase 3: output projection ----------------
                for tb in range(16):
                    for nk in range(2):
                        py = psa.tile([128, 512], F32, tag="mm")
                        for cp in range(2):
                            nc.tensor.matmul(
                                py[:],
                                oT_sb[:, cp, tb * 128:(tb + 1) * 128],
                                wo_sb[:, cp, nk * 512:(nk + 1) * 512],
                                start=(cp == 0),
                                stop=(cp == 1),
                            )
                        yt = ypool.tile([128, 512], F32, tag="yt")
                        if nk == 0:
                            nc.scalar.copy(yt[:], py[:])
                        else:
                            nc.vector.tensor_copy(yt[:], py[:])
                        nc.sync.dma_start(
                            y[tb * 128:(tb + 1) * 128, nk * 512:(nk + 1) * 512],
                            yt[:],
                        )

    nc.compile()
    return nc


def _get_nc():
    global _compiled
    if _compiled is None:
        _compiled = _build()
    return _compiled


class _Runner:
    """Compiled PJRT executor for the SPMD kernel, reusable across calls."""

    def __init__(self, nc):
        import jax
        import concourse.mybir as mybir
        from concourse import bass2jax
        from jax.experimental.shard_map import shard_map
        from jax.sharding import Mesh, PartitionSpec

        self.jax = jax
        self.nc = nc
        bass2jax.install_neuronx_cc_hook()

        partition_name = (
            nc.partition_id_tensor.name if nc.partition_id_tensor else None
        )
        in_names, out_names, out_avals, zero_outs = [], [], [], []
        for alloc in nc.m.functions[0].allocations:
            if not isinstance(alloc, mybir.MemoryLocationSet):
                continue
            name = alloc.memorylocations[0].name
            if alloc.kind == "ExternalInput":
                if name != partition_name:
                    in_names.append(name)
            elif alloc.kind == "ExternalOutput":
                out_names.append(name)
                shape = tuple(alloc.tensor_shape)
                dtype = mybir.dt.np(alloc.dtype)
                out_avals.append(jax.core.ShapedArray(shape, dtype))
                zero_outs.append(np.zeros(shape, dtype))
        self.in_names = in_names
        self.out_names = out_names
        self.out_avals = out_avals
        self.zero_outs = zero_outs
        all_names = tuple(in_names + out_names)

        if partition_name is not None:
            all_names = all_names + (partition_name,)

        def _body(*args):
            operands = list(args)
            if partition_name is not None:
                operands.append(bass2jax.partition_id_tensor())
            outs = bass2jax._bass_exec_p.bind(
                *operands,
                out_avals=tuple(out_avals),
                in_names=all_names,
                out_names=tuple(out_names),
                lowering_input_output_aliases=(),
                sim_require_finite=True,
                sim_require_nnan=True,
                nc=nc,
            )
            return tuple(outs)

        devices = jax.devices()[:NCORES]
        assert len(devices) == NCORES
        mesh = Mesh(np.asarray(devices), ("core",))
        n_args = len(in_names) + len(out_names)
        self.fn = jax.jit(
            shard_map(
                _body,
                mesh=mesh,
                in_specs=(PartitionSpec("core"),) * n_args,
                out_specs=(PartitionSpec("core"),) * len(out_names),
                check_rep=False,
            ),
            keep_unused=True,
        )

    def device_args(self, in_maps):
        args = [
            np.concatenate([np.asarray(m[name]) for m in in_maps], axis=0)
            for name in self.in_names
        ]
        args += [
            np.zeros((NCORES * z.shape[0], *z.shape[1:]), z.dtype)
            for z in self.zero_outs
        ]
        return [self.jax.device_put(a) for a in args]

    def run_device(self, dev_args):
        return self.fn(*dev_args)

    def run(self, in_maps):
        out_arrs = self.fn(*self.device_args(in_maps))
        return [
            {
                name: np.asarray(out_arrs[i]).reshape(
                    NCORES, *self.out_avals[i].shape
                )[c]
                for i, name in enumerate(self.out_names)
            }
            for c in range(NCORES)
        ]


_runner = None


def _get_runner():
    global _runner
    if _runner is None:
        _runner = _Runner(_get_nc())
    return _runner


def make_in_maps(x, Wqkv, Wo):
    x = np.asarray(x, dtype=np.float32)
    Wqkv = np.asarray(Wqkv, dtype=np.float32)
    Wo = np.asarray(Wo, dtype=np.float32)
    mask = np.triu(np.ones((128, 128), dtype=np.float32))
    in_maps = []
    for c in range(NCORES):
        b, g = c // 4, c % 4
        in_maps.append({
            "xT": np.ascontiguousarray(x[b].T),
            "wq": np.ascontiguousarray(Wqkv[:, g * CO:(g + 1) * CO]),
            "wk": np.ascontiguousarray(Wqkv[:, C + g * CO:C + (g + 1) * CO]),
            "wv": np.ascontiguousarray(Wqkv[:, 2 * C + g * CO:2 * C + (g + 1) * CO]),
            "wo": np.ascontiguousarray(Wo[g * CO:(g + 1) * CO, :]),
            "mask": mask,
        })
    return in_maps


def gather_output(results):
    y = np.zeros((B, T, C), dtype=np.float32)
    for c in range(NCORES):
        y[c // 4] += results[c]["y"]
    return y


def kernel(x, Wqkv, Wo):
    runner = _get_runner()
    in_maps = make_in_maps(x, Wqkv, Wo)
    return gather_output(runner.run(in_maps))
